# revision 3
# baseline (speedup 1.0000x reference)
"""Trainium2 Bass kernel for nn_AttentionEncoder (ragged_sequence).

Math collapse used here (validated vs the jax reference to ~1e-7):
  - k comes from exercise embeddings -> batch-independent K [S,D].
  - q/v come from resp_emb[p] with p in {0,1,2}; valid positions have
    p in {1,2}, so per batch the softmax rows take only 2 distinct
    values. The whole [B,H,S,S] attention collapses to
        E[r,h,t]   = exp(score(q_r, k_t) / sqrt(dh))          (batch-free)
        A[r,h,j,b] = sum_t E[r,h,t] * (p[b,t]==j)             (tiny matmul)
    and the masked-mean-pooled output becomes
        pooled[b,d] = G1[h(d),b]*v1[d] + G2[h(d),b]*v2[d]
    with G linear in A. Final: sigmoid(pooled @ Wmap + bmap).

Sharding: data-parallel over batch. Each of the 8 cores gets B/8=16 rows
of p_matrix; all (small) tables/weights are replicated. No collectives.
"""

import sys

sys.path.insert(0, "/opt/trn_rl_repo")

import math
from contextlib import ExitStack

import numpy as np

import concourse.bass as bass
import concourse.tile as tile
from concourse import bacc, masks, mybir
from concourse.bass_utils import run_bass_kernel_spmd

B, S, D, C, O, H = 128, 512, 128, 128, 256, 4
DH = D // H  # 32
NCORES = 8
BS = B // NCORES  # 16 batch rows per core
KT = S // 128  # 4 sequence tiles

F32 = mybir.dt.float32
I32 = mybir.dt.int32
AF = mybir.ActivationFunctionType
ALU = mybir.AluOpType
AX = mybir.AxisListType


def build_kernel(ctx: ExitStack, tc: tile.TileContext, io: dict):
    nc = tc.nc
    sb = ctx.enter_context(tc.tile_pool(name="sb", bufs=1))
    ps = ctx.enter_context(tc.tile_pool(name="ps", bufs=4, space="PSUM"))
    psa = ctx.enter_context(tc.tile_pool(name="psa", bufs=1, space="PSUM"))

    ident = sb.tile([128, 128], F32, tag="ident")
    masks.make_identity(nc, ident[:])

    # ---- DMA inputs ----
    p_sb = sb.tile([BS, S], I32, tag="p")
    nc.gpsimd.dma_start(p_sb[:], io["p_matrix"].ap())
    q_sb = sb.tile([128, KT, C], F32, tag="q")  # Q rows s -> [s%128, s//128, c]
    nc.gpsimd.dma_start(q_sb[:], io["Q_matrix"].ap().rearrange("(k p) c -> p k c", p=128))
    ee_sb = sb.tile([128, KT, D], F32, tag="ee")
    nc.gpsimd.dma_start(ee_sb[:], io["exer_emb"].ap().rearrange("(k p) d -> p k d", p=128))
    lam_sb = sb.tile([128, KT], F32, tag="lam")
    nc.gpsimd.dma_start(lam_sb[:], io["exer_lam"].ap().rearrange("(k p) one -> p (k one)", p=128))
    ce_sb = sb.tile([C, D], F32, tag="ce")
    nc.gpsimd.dma_start(ce_sb[:], io["concept_emb"].ap())
    ret_sb = sb.tile([D, 3], F32, tag="ret")  # resp_emb^T
    nc.gpsimd.dma_start(ret_sb[:], io["resp_emb"].ap().rearrange("r d -> d r"))
    wq_sb = sb.tile([D, D], F32, tag="wq")
    nc.gpsimd.dma_start(wq_sb[:], io["Wq"].ap())
    wk_sb = sb.tile([D, D], F32, tag="wk")
    nc.gpsimd.dma_start(wk_sb[:], io["Wk"].ap())
    wv_sb = sb.tile([D, D], F32, tag="wv")
    nc.gpsimd.dma_start(wv_sb[:], io["Wv"].ap())
    bq_sb = sb.tile([D, 1], F32, tag="bq")
    nc.gpsimd.dma_start(bq_sb[:], io["bq"].ap().unsqueeze(1))
    bk_sb = sb.tile([D, 1], F32, tag="bk")
    nc.gpsimd.dma_start(bk_sb[:], io["bk"].ap().unsqueeze(1))
    bv_sb = sb.tile([D, 1], F32, tag="bv")
    nc.gpsimd.dma_start(bv_sb[:], io["bv"].ap().unsqueeze(1))
    wm_sb = sb.tile([D, O], F32, tag="wm")
    nc.gpsimd.dma_start(wm_sb[:], io["Wmap"].ap())
    bmt_sb = sb.tile([128, O // 128], F32, tag="bmt")  # bmap as [o%128, o//128]
    nc.gpsimd.dma_start(bmt_sb[:], io["bmap"].ap().rearrange("(t p) -> p t", p=128))

    # ---- batch-independent preprocessing: K^T tiles [d, t] ----
    kt_all = sb.tile([D, KT, 128], F32, tag="kt_all")
    for k in range(KT):
        qt_ps = ps.tile([128, 128], F32, tag="ps")
        nc.tensor.transpose(qt_ps[:], q_sb[:, k, :], ident[:])  # Q^T tile [c, s]
        qt_sb = sb.tile([128, 128], F32, tag=f"qt{k}")
        nc.scalar.copy(qt_sb[:], qt_ps[:])
        cm_ps = ps.tile([128, 128], F32, tag="ps")  # concept part [s, d]
        nc.tensor.matmul(cm_ps[:], qt_sb[:], ce_sb[:])
        rs_sb = sb.tile([128, 1], F32, tag=f"rs{k}")
        nc.vector.reduce_sum(rs_sb[:], q_sb[:, k, :], axis=AX.X)
        rr_sb = sb.tile([128, 1], F32, tag=f"rr{k}")
        nc.vector.reciprocal(rr_sb[:], rs_sb[:])
        sc_sb = sb.tile([128, 1], F32, tag=f"sc{k}")
        nc.vector.tensor_mul(sc_sb[:], rr_sb[:], lam_sb[:, k : k + 1])
        ex_sb = sb.tile([128, 128], F32, tag=f"ex{k}")  # ex tile [s, d]
        nc.vector.scalar_tensor_tensor(
            ex_sb[:], cm_ps[:], sc_sb[:], ee_sb[:, k, :], op0=ALU.mult, op1=ALU.add
        )
        ext_ps = ps.tile([128, 128], F32, tag="ps")
        nc.tensor.transpose(ext_ps[:], ex_sb[:], ident[:])  # ex^T [d, s]
        ext_sb = sb.tile([128, 128], F32, tag=f"ext{k}")
        nc.scalar.copy(ext_sb[:], ext_ps[:])
        kt_ps = ps.tile([128, 128], F32, tag="ps")  # K^T tile [d, t]
        nc.tensor.matmul(kt_ps[:], wk_sb[:], ext_sb[:])
        nc.scalar.add(kt_all[:, k, :], kt_ps[:], bk_sb[:])

    # ---- q/v tables ----
    qt3_ps = ps.tile([D, 3], F32, tag="ps")
    nc.tensor.matmul(qt3_ps[:], wq_sb[:], ret_sb[:])
    qt3_sb = sb.tile([D, 3], F32, tag="qt3")
    nc.scalar.add(qt3_sb[:], qt3_ps[:], bq_sb[:])
    vt3_ps = ps.tile([D, 3], F32, tag="ps")
    nc.tensor.matmul(vt3_ps[:], wv_sb[:], ret_sb[:])
    vt3_sb = sb.tile([D, 3], F32, tag="vt3")
    nc.scalar.add(vt3_sb[:], vt3_ps[:], bv_sb[:])
    vf_ps = ps.tile([3, 128], F32, tag="ps")
    nc.tensor.transpose(vf_ps[:], vt3_sb[:], ident[:])  # v rows [r, d]
    vf_sb = sb.tile([3, 128], F32, tag="vf")
    nc.scalar.copy(vf_sb[:], vf_ps[:])

    # qpad^T [d, rh]: column r*4+h holds q_{r+1} masked to head h
    qpad_sb = sb.tile([D, 8], F32, tag="qpad")
    nc.gpsimd.memset(qpad_sb[:], 0.0)
    for r in range(2):
        for h in range(H):
            nc.vector.tensor_copy(
                qpad_sb[h * DH : (h + 1) * DH, r * 4 + h : r * 4 + h + 1],
                qt3_sb[h * DH : (h + 1) * DH, 1 + r : 2 + r],
            )
    # vsel [j*4+h, d]: row holds v_{j+1} masked to head h.
    # Compute engines need partition starts at 0/32/64/96, so place these
    # single-row cross-partition copies with SBUF->SBUF DMAs instead.
    vsel_sb = sb.tile([8, 128], F32, tag="vsel")
    nc.gpsimd.memset(vsel_sb[:], 0.0)
    for j in range(2):
        for h in range(H):
            nc.gpsimd.dma_start(
                vsel_sb[j * 4 + h : j * 4 + h + 1, h * DH : (h + 1) * DH],
                vf_sb[1 + j : 2 + j, h * DH : (h + 1) * DH],
            )

    # ---- E = exp(scores) [t, rh] per tile ----
    e_all = sb.tile([128, KT, 8], F32, tag="e_all")
    for k in range(KT):
        st_ps = ps.tile([128, 8], F32, tag="ps")
        nc.tensor.matmul(st_ps[:], kt_all[:, k, :], qpad_sb[:])
        nc.scalar.activation(e_all[:, k, :], st_ps[:], AF.Exp, scale=1.0 / math.sqrt(DH))

    # ---- masks, counts ----
    m1_sb = sb.tile([BS, S], F32, tag="m1")
    nc.vector.tensor_scalar(m1_sb[:], p_sb[:], 1, None, op0=ALU.is_equal)
    m2_sb = sb.tile([BS, S], F32, tag="m2")
    nc.vector.tensor_scalar(m2_sb[:], p_sb[:], 2, None, op0=ALU.is_equal)
    c1_sb = sb.tile([BS, 1], F32, tag="c1")
    nc.vector.reduce_sum(c1_sb[:], m1_sb[:], axis=AX.X)
    c2_sb = sb.tile([BS, 1], F32, tag="c2")
    nc.vector.reduce_sum(c2_sb[:], m2_sb[:], axis=AX.X)
    ct_sb = sb.tile([BS, 1], F32, tag="ct")
    nc.vector.tensor_add(ct_sb[:], c1_sb[:], c2_sb[:])
    ctc_sb = sb.tile([BS, 1], F32, tag="ctc")
    nc.vector.tensor_scalar_max(ctc_sb[:], ct_sb[:], 1.0)
    rc_sb = sb.tile([BS, 1], F32, tag="rc")
    nc.vector.reciprocal(rc_sb[:], ctc_sb[:])
    c1n_sb = sb.tile([BS, 1], F32, tag="c1n")
    nc.vector.tensor_mul(c1n_sb[:], c1_sb[:], rc_sb[:])
    c2n_sb = sb.tile([BS, 1], F32, tag="c2n")
    nc.vector.tensor_mul(c2n_sb[:], c2_sb[:], rc_sb[:])

    # ---- Mstack tiles [t, j*32+b] and A accumulation ----
    # j blocks sit at columns 0 and 32 so the A halves land on partition
    # starts 0 and 32 (compute-engine partition-start constraint).
    ms_all = sb.tile([128, KT, 64], F32, tag="ms_all")
    nc.gpsimd.memset(ms_all[:], 0.0)
    for k in range(KT):
        for j, m_sb in enumerate((m1_sb, m2_sb)):
            tp_ps = ps.tile([128, BS], F32, tag="ps")
            nc.tensor.transpose(tp_ps[:], m_sb[:, k * 128 : (k + 1) * 128], ident[:BS, :BS])
            nc.scalar.copy(ms_all[:, k, j * 32 : j * 32 + BS], tp_ps[:])

    a_ps = psa.tile([64, 8], F32, tag="a_ps")  # A[j*32+b, r*4+h]
    for k in range(KT):
        nc.tensor.matmul(
            a_ps[:], ms_all[:, k, :], e_all[:, k, :], start=(k == 0), stop=(k == KT - 1)
        )
    a1_sb = sb.tile([BS, 8], F32, tag="a1_sb")
    nc.scalar.copy(a1_sb[:], a_ps[0:BS, :])
    a2_sb = sb.tile([BS, 8], F32, tag="a2_sb")
    nc.scalar.copy(a2_sb[:], a_ps[32 : 32 + BS, :])

    # ---- G weights [b, j*4+h] ----
    z_sb = sb.tile([BS, 8], F32, tag="z")
    nc.vector.tensor_add(z_sb[:], a1_sb[:], a2_sb[:])
    rz_sb = sb.tile([BS, 8], F32, tag="rz")
    nc.vector.reciprocal(rz_sb[:], z_sb[:])
    w0_sb = sb.tile([BS, 8], F32, tag="w0")  # mass on v1, cols r*4+h
    nc.vector.tensor_mul(w0_sb[:], a1_sb[:], rz_sb[:])
    w1_sb = sb.tile([BS, 8], F32, tag="w1")  # mass on v2
    nc.vector.tensor_mul(w1_sb[:], a2_sb[:], rz_sb[:])
    gcat_sb = sb.tile([BS, 8], F32, tag="gcat")
    tmp0_sb = sb.tile([BS, 4], F32, tag="tmp0")
    nc.vector.tensor_scalar_mul(tmp0_sb[:], w0_sb[:, 4:8], c2n_sb[:])
    nc.vector.scalar_tensor_tensor(
        gcat_sb[:, 0:4], w0_sb[:, 0:4], c1n_sb[:], tmp0_sb[:], op0=ALU.mult, op1=ALU.add
    )
    tmp1_sb = sb.tile([BS, 4], F32, tag="tmp1")
    nc.vector.tensor_scalar_mul(tmp1_sb[:], w1_sb[:, 4:8], c2n_sb[:])
    nc.vector.scalar_tensor_tensor(
        gcat_sb[:, 4:8], w1_sb[:, 0:4], c1n_sb[:], tmp1_sb[:], op0=ALU.mult, op1=ALU.add
    )

    # ---- pooled and output ----
    gt_ps = ps.tile([8, BS], F32, tag="ps")
    nc.tensor.transpose(gt_ps[:], gcat_sb[:], ident[:BS, :BS])
    gt_sb = sb.tile([8, BS], F32, tag="gt")
    nc.scalar.copy(gt_sb[:], gt_ps[:])
    pooled_ps = ps.tile([BS, D], F32, tag="ps")
    nc.tensor.matmul(pooled_ps[:], gt_sb[:], vsel_sb[:])
    pooled_sb = sb.tile([BS, D], F32, tag="pooled")
    nc.scalar.copy(pooled_sb[:], pooled_ps[:])
    pt_ps = ps.tile([D, BS], F32, tag="ps")
    nc.tensor.transpose(pt_ps[:], pooled_sb[:], ident[:BS, :BS])
    pt_sb = sb.tile([D, BS], F32, tag="pt")
    nc.scalar.copy(pt_sb[:], pt_ps[:])

    out_sb = sb.tile([BS, O], F32, tag="out_sb")
    for t in range(O // 128):
        o_ps = ps.tile([128, BS], F32, tag="ps")
        nc.tensor.matmul(o_ps[:], wm_sb[:, t * 128 : (t + 1) * 128], pt_sb[:])
        osig_sb = sb.tile([128, BS], F32, tag=f"osig{t}")
        nc.scalar.activation(osig_sb[:], o_ps[:], AF.Sigmoid, bias=bmt_sb[:, t : t + 1])
        ot_ps = ps.tile([BS, 128], F32, tag="ps")
        nc.tensor.transpose(ot_ps[:], osig_sb[:], ident[:])
        nc.scalar.copy(out_sb[:, t * 128 : (t + 1) * 128], ot_ps[:])
    nc.gpsimd.dma_start(io["out"].ap(), out_sb[:])


INPUT_SPECS = {
    "p_matrix": ([BS, S], I32),
    "exer_emb": ([S, D], F32),
    "exer_lam": ([S, 1], F32),
    "concept_emb": ([C, D], F32),
    "resp_emb": ([3, D], F32),
    "Q_matrix": ([S, C], F32),
    "Wq": ([D, D], F32),
    "bq": ([D], F32),
    "Wk": ([D, D], F32),
    "bk": ([D], F32),
    "Wv": ([D, D], F32),
    "bv": ([D], F32),
    "Wmap": ([D, O], F32),
    "bmap": ([O], F32),
}

_cache = {}


def build_nc():
    if "nc" in _cache:
        return _cache["nc"]
    nc = bacc.Bacc("TRN2", target_bir_lowering=False, debug=False, num_devices=NCORES)
    io = {}
    for name, (shape, dt) in INPUT_SPECS.items():
        io[name] = nc.dram_tensor(name, shape, dt, kind="ExternalInput")
    io["out"] = nc.dram_tensor("out", [BS, O], F32, kind="ExternalOutput")
    with tile.TileContext(nc) as tc:
        with ExitStack() as ctx:
            build_kernel(ctx, tc, io)
    nc.compile()
    _cache["nc"] = nc
    return nc


def make_in_maps(inputs: dict) -> list:
    in_maps = []
    for i in range(NCORES):
        m = {}
        for name in INPUT_SPECS:
            arr = np.ascontiguousarray(inputs[name])
            if name == "p_matrix":
                arr = np.ascontiguousarray(arr[i * BS : (i + 1) * BS])
            m[name] = arr
        in_maps.append(m)
    return in_maps


def kernel(**inputs) -> np.ndarray:
    nc = build_nc()
    in_maps = make_in_maps(inputs)
    res = run_bass_kernel_spmd(nc, in_maps, core_ids=list(range(NCORES)))
    return np.concatenate([res.results[i]["out"] for i in range(NCORES)], axis=0)


if __name__ == "__main__":
    rng = np.random.default_rng(0)
    fake = {
        "p_matrix": rng.integers(0, 3, (B, S), dtype=np.int32),
        "exer_emb": rng.normal(size=(S, D)).astype(np.float32) * 0.02,
        "exer_lam": rng.normal(size=(S, 1)).astype(np.float32) * 0.02,
        "concept_emb": rng.normal(size=(C, D)).astype(np.float32) * 0.02,
        "resp_emb": rng.normal(size=(3, D)).astype(np.float32) * 0.02,
        "Q_matrix": (rng.random((S, C)) < 0.1).astype(np.float32),
        "Wq": rng.normal(size=(D, D)).astype(np.float32) * 0.02,
        "bq": np.zeros(D, np.float32),
        "Wk": rng.normal(size=(D, D)).astype(np.float32) * 0.02,
        "bk": np.zeros(D, np.float32),
        "Wv": rng.normal(size=(D, D)).astype(np.float32) * 0.02,
        "bv": np.zeros(D, np.float32),
        "Wmap": rng.normal(size=(D, O)).astype(np.float32) * 0.02,
        "bmap": np.zeros(O, np.float32),
    }
    out = kernel(**fake)
    print(out.shape, out.dtype)


# revision 5
# speedup vs baseline: 1.6638x; 1.6638x over previous
"""Trainium2 Bass kernel for nn_AttentionEncoder (ragged_sequence).

Math collapse used here (validated vs the jax reference to ~1e-7):
  - k comes from exercise embeddings -> batch-independent K [S,D].
  - q/v come from resp_emb[p] with p in {0,1,2}; valid positions have
    p in {1,2}, so per batch the softmax rows take only 2 distinct
    values. The whole [B,H,S,S] attention collapses to
        E[r,h,t]   = exp(score(q_r, k_t) / sqrt(dh))          (batch-free)
        A[r,h,j,b] = sum_t E[r,h,t] * (p[b,t]==j)             (tiny matmul)
    and the masked-mean-pooled output becomes
        pooled[b,d] = G1[h(d),b]*v1[d] + G2[h(d),b]*v2[d]
    with G linear in A. Final: sigmoid(pooled @ Wmap + bmap).

Sharding: data-parallel over batch. Each of the 8 cores gets B/8=16 rows
of p_matrix; all (small) tables/weights are replicated. No collectives.

The score path runs in bf16 (scores are ~1e-4, exp(score)~1, and the
output tolerance is 2e-2; bf16 error on this path is ~1e-5 at the
output). The value/count path that actually determines the output stays
f32: E, the indicator matmuls, G, and the sigmoid epilogue.
"""

import sys

sys.path.insert(0, "/opt/trn_rl_repo")

import math
from contextlib import ExitStack

import ml_dtypes
import numpy as np

import concourse.bass as bass
import concourse.tile as tile
from concourse import bacc, masks, mybir
from concourse.bass_utils import run_bass_kernel_spmd

B, S, D, C, O, H = 128, 512, 128, 128, 256, 4
DH = D // H  # 32
NCORES = 8
BS = B // NCORES  # 16 batch rows per core
KT = S // 128  # 4 sequence tiles

F32 = mybir.dt.float32
BF16 = mybir.dt.bfloat16
AF = mybir.ActivationFunctionType
ALU = mybir.AluOpType
AX = mybir.AxisListType


def build_kernel(ctx: ExitStack, tc: tile.TileContext, io: dict):
    nc = tc.nc
    sb = ctx.enter_context(tc.tile_pool(name="sb", bufs=1))
    ps = ctx.enter_context(tc.tile_pool(name="ps", bufs=4, space="PSUM"))
    psb = ctx.enter_context(tc.tile_pool(name="psb", bufs=2, space="PSUM"))
    psa = ctx.enter_context(tc.tile_pool(name="psa", bufs=1, space="PSUM"))

    ident = sb.tile([128, 128], BF16, tag="ident")
    masks.make_identity(nc, ident[:])

    # ---- DMA inputs (HWDGE via sync engine; hottest first) ----
    q_sb = sb.tile([128, KT, C], BF16, tag="q")  # Q rows s -> [s%128, s//128, c]
    nc.sync.dma_start(q_sb[:], io["Q_matrix"].ap().rearrange("(k p) c -> p k c", p=128))
    p_sb = sb.tile([BS, S], BF16, tag="p")
    nc.sync.dma_start(p_sb[:], io["p_matrix"].ap())
    lam_sb = sb.tile([128, KT], F32, tag="lam")
    nc.sync.dma_start(lam_sb[:], io["exer_lam"].ap().rearrange("(k p) one -> p (k one)", p=128))
    ce_sb = sb.tile([C, D], BF16, tag="ce")
    nc.sync.dma_start(ce_sb[:], io["concept_emb"].ap())
    ee_sb = sb.tile([128, KT, D], BF16, tag="ee")
    nc.sync.dma_start(ee_sb[:], io["exer_emb"].ap().rearrange("(k p) d -> p k d", p=128))
    ret_sb = sb.tile([D, 3], F32, tag="ret")  # resp_emb^T
    nc.sync.dma_start(ret_sb[:], io["resp_emb"].ap().rearrange("r d -> d r"))
    wk_sb = sb.tile([D, D], BF16, tag="wk")
    nc.sync.dma_start(wk_sb[:], io["Wk"].ap())
    wq_sb = sb.tile([D, D], F32, tag="wq")
    nc.sync.dma_start(wq_sb[:], io["Wq"].ap())
    wv_sb = sb.tile([D, D], F32, tag="wv")
    nc.sync.dma_start(wv_sb[:], io["Wv"].ap())
    bq_sb = sb.tile([D, 1], F32, tag="bq")
    nc.sync.dma_start(bq_sb[:], io["bq"].ap().unsqueeze(1))
    bk_sb = sb.tile([D, 1], F32, tag="bk")
    nc.sync.dma_start(bk_sb[:], io["bk"].ap().unsqueeze(1))
    bv_sb = sb.tile([D, 1], F32, tag="bv")
    nc.sync.dma_start(bv_sb[:], io["bv"].ap().unsqueeze(1))
    wm_sb = sb.tile([D, O], BF16, tag="wm")
    nc.sync.dma_start(wm_sb[:], io["Wmap"].ap())
    bmapb_sb = sb.tile([BS, O], F32, tag="bmapb")  # bmap broadcast over partitions
    nc.sync.dma_start(bmapb_sb[:], io["bmap"].ap().unsqueeze(0).to_broadcast([BS, O]))

    # ---- K^T [d, t] in one batched matmul chain ----
    # ex = ee + (lam/rowsum(Q)) * (Q @ ce); fold the row scale into Q first
    # so (lam*cm)^T comes out of a single ce-stationary matmul.
    rsum_sb = sb.tile([128, KT], F32, tag="rsum")
    nc.vector.reduce_sum(rsum_sb[:], q_sb[:], axis=AX.X)
    rrec_sb = sb.tile([128, KT], F32, tag="rrec")
    nc.vector.reciprocal(rrec_sb[:], rsum_sb[:])
    sc_sb = sb.tile([128, KT], F32, tag="sc")
    nc.vector.tensor_mul(sc_sb[:], rrec_sb[:], lam_sb[:])
    qs_sb = sb.tile([128, KT, C], BF16, tag="qs")
    for k in range(KT):
        nc.vector.tensor_scalar_mul(qs_sb[:, k, :], q_sb[:, k, :], sc_sb[:, k : k + 1])
    qst_sb = sb.tile([C, S], BF16, tag="qst")  # (scaled Q)^T [c, s]
    eet_sb = sb.tile([D, S], BF16, tag="eet")  # ee^T [d, s]
    for k in range(KT):
        qst_ps = ps.tile([128, 128], BF16, tag="ps")
        nc.tensor.transpose(qst_ps[:], qs_sb[:, k, :], ident[:])
        if k % 2:
            nc.scalar.copy(qst_sb[:, k * 128 : (k + 1) * 128], qst_ps[:])
        else:
            nc.vector.tensor_copy(qst_sb[:, k * 128 : (k + 1) * 128], qst_ps[:])
        eet_ps = ps.tile([128, 128], BF16, tag="ps")
        nc.tensor.transpose(eet_ps[:], ee_sb[:, k, :], ident[:])
        if k % 2:
            nc.scalar.copy(eet_sb[:, k * 128 : (k + 1) * 128], eet_ps[:])
        else:
            nc.vector.tensor_copy(eet_sb[:, k * 128 : (k + 1) * 128], eet_ps[:])
    cm_ps = psb.tile([128, S], F32, tag="big")  # (lam*cm)^T
    nc.tensor.matmul(cm_ps[:], ce_sb[:], qst_sb[:])
    ext_sb = sb.tile([D, S], BF16, tag="ext")  # ex^T
    nc.vector.tensor_add(ext_sb[:], cm_ps[:], eet_sb[:])
    kt_ps = psb.tile([128, S], F32, tag="big")
    nc.tensor.matmul(kt_ps[:], wk_sb[:], ext_sb[:])
    kt_sb = sb.tile([D, S], BF16, tag="kt")  # K^T [d, t] with bias
    nc.vector.tensor_scalar_add(kt_sb[:], kt_ps[:], bk_sb[:])

    # ---- q/v tables ----
    qt_ps = ps.tile([D, 3], F32, tag="ps")
    nc.tensor.matmul(qt_ps[:], wq_sb[:], ret_sb[:])
    qt_sb = sb.tile([D, 3], BF16, tag="qt3")
    nc.scalar.add(qt_sb[:], qt_ps[:], bq_sb[:])
    vt_ps = ps.tile([D, 3], F32, tag="ps")
    nc.tensor.matmul(vt_ps[:], wv_sb[:], ret_sb[:])
    vt_sb = sb.tile([D, 3], BF16, tag="vt3")
    nc.scalar.add(vt_sb[:], vt_ps[:], bv_sb[:])
    vf_ps = ps.tile([3, 128], BF16, tag="ps")
    nc.tensor.transpose(vf_ps[:], vt_sb[:], ident[:])
    vf_sb = sb.tile([3, 128], BF16, tag="vf")
    nc.vector.tensor_copy(vf_sb[:], vf_ps[:])

    # qpad^T [d, rh]: column r*4+h holds q_{r+1} masked to head h
    qpad_sb = sb.tile([D, 8], BF16, tag="qpad")
    nc.gpsimd.memset(qpad_sb[:], 0.0)
    for r in range(2):
        for h in range(H):
            nc.vector.tensor_copy(
                qpad_sb[h * DH : (h + 1) * DH, r * 4 + h : r * 4 + h + 1],
                qt_sb[h * DH : (h + 1) * DH, 1 + r : 2 + r],
            )
    # vsel [j*4+h, d]: row holds v_{j+1} masked to head h. Single-row
    # cross-partition placement -> tiny SBUF->SBUF DMAs (no partition-start
    # constraint there).
    vsel_sb = sb.tile([8, 128], BF16, tag="vsel")
    nc.gpsimd.memset(vsel_sb[:], 0.0)
    for j in range(2):
        for h in range(H):
            nc.sync.dma_start(
                vsel_sb[j * 4 + h : j * 4 + h + 1, h * DH : (h + 1) * DH],
                vf_sb[1 + j : 2 + j, h * DH : (h + 1) * DH],
            )

    # ---- E = exp(scores) [t, rh] per tile; col 8 stays 1.0 for counts ----
    e_all = sb.tile([128, KT, 9], F32, tag="e_all")
    nc.gpsimd.memset(e_all[:], 1.0)
    for k in range(KT):
        st_ps = ps.tile([128, 8], F32, tag="ps")
        nc.tensor.matmul(st_ps[:], kt_sb[:, k * 128 : (k + 1) * 128], qpad_sb[:])
        nc.scalar.activation(
            e_all[:, k, 0:8], st_ps[:], AF.Exp, scale=1.0 / math.sqrt(DH)
        )

    # ---- transposed p -> indicator stacks [t, j*32+b] ----
    pt_sb = sb.tile([128, KT, BS], BF16, tag="pt")
    for k in range(KT):
        pp_ps = ps.tile([128, BS], BF16, tag="ps")
        nc.tensor.transpose(pp_ps[:], p_sb[:, k * 128 : (k + 1) * 128], ident[:BS, :BS])
        if k % 2:
            nc.scalar.copy(pt_sb[:, k, :], pp_ps[:])
        else:
            nc.vector.tensor_copy(pt_sb[:, k, :], pp_ps[:])
    # j blocks at columns 0 and 32 so A halves land on partition starts 0/32
    ms_all = sb.tile([128, KT, 64], F32, tag="ms_all")
    nc.gpsimd.memset(ms_all[:], 0.0)
    nc.vector.tensor_scalar(ms_all[:, :, 0:BS], pt_sb[:], 1.0, None, op0=ALU.is_equal)
    nc.vector.tensor_scalar(
        ms_all[:, :, 32 : 32 + BS], pt_sb[:], 2.0, None, op0=ALU.is_equal
    )

    # ---- A[j*32+b, r*4+h] plus counts in col 8 ----
    a_ps = psa.tile([64, 9], F32, tag="a_ps")
    for k in range(KT):
        nc.tensor.matmul(
            a_ps[:], ms_all[:, k, :], e_all[:, k, :], start=(k == 0), stop=(k == KT - 1)
        )
    a1_sb = sb.tile([BS, 9], F32, tag="a1_sb")
    nc.vector.tensor_copy(a1_sb[:], a_ps[0:BS, :])
    a2_sb = sb.tile([BS, 9], F32, tag="a2_sb")
    nc.vector.tensor_copy(a2_sb[:], a_ps[32 : 32 + BS, :])

    # ---- G weights [b, j*4+h] ----
    z_sb = sb.tile([BS, 9], F32, tag="z")  # col 8 = total count
    nc.vector.tensor_add(z_sb[:], a1_sb[:], a2_sb[:])
    zc_sb = sb.tile([BS, 9], F32, tag="zc")
    nc.vector.tensor_scalar_max(zc_sb[:], z_sb[:], 1e-30)
    rz_sb = sb.tile([BS, 9], F32, tag="rz")
    nc.vector.reciprocal(rz_sb[:], zc_sb[:])
    w0_sb = sb.tile([BS, 9], F32, tag="w0")  # mass on v1 (col 8 = c1n)
    nc.vector.tensor_mul(w0_sb[:], a1_sb[:], rz_sb[:])
    w1_sb = sb.tile([BS, 9], F32, tag="w1")  # mass on v2 (col 8 = c2n)
    nc.vector.tensor_mul(w1_sb[:], a2_sb[:], rz_sb[:])
    gcat_sb = sb.tile([BS, 8], BF16, tag="gcat")
    tmp0_sb = sb.tile([BS, 4], F32, tag="tmp0")
    nc.vector.tensor_scalar_mul(tmp0_sb[:], w0_sb[:, 4:8], w1_sb[:, 8:9])
    nc.vector.scalar_tensor_tensor(
        gcat_sb[:, 0:4], w0_sb[:, 0:4], w0_sb[:, 8:9], tmp0_sb[:], op0=ALU.mult, op1=ALU.add
    )
    tmp1_sb = sb.tile([BS, 4], F32, tag="tmp1")
    nc.vector.tensor_scalar_mul(tmp1_sb[:], w1_sb[:, 4:8], w1_sb[:, 8:9])
    nc.vector.scalar_tensor_tensor(
        gcat_sb[:, 4:8], w1_sb[:, 0:4], w0_sb[:, 8:9], tmp1_sb[:], op0=ALU.mult, op1=ALU.add
    )

    # ---- pooled^T directly, then the output matmul ----
    gt_ps = ps.tile([8, BS], BF16, tag="ps")
    nc.tensor.transpose(gt_ps[:], gcat_sb[:], ident[:BS, :BS])
    gt_sb = sb.tile([8, BS], BF16, tag="gt")
    nc.vector.tensor_copy(gt_sb[:], gt_ps[:])
    ptd_ps = ps.tile([D, BS], F32, tag="ps")
    nc.tensor.matmul(ptd_ps[:], vsel_sb[:], gt_sb[:])  # vsel.T @ gt = pooled^T
    pt2_sb = sb.tile([D, BS], BF16, tag="pt2")
    nc.vector.tensor_copy(pt2_sb[:], ptd_ps[:])
    o_ps = ps.tile([BS, O], F32, tag="ps")
    nc.tensor.matmul(o_ps[:], pt2_sb[:], wm_sb[:])
    logit_sb = sb.tile([BS, O], F32, tag="logit")
    nc.vector.tensor_add(logit_sb[:], o_ps[:], bmapb_sb[:])
    # sigmoid(x) = 1/(1+exp(-x)) using the Exp table already loaded
    eneg_sb = sb.tile([BS, O], F32, tag="eneg")
    nc.scalar.activation(eneg_sb[:], logit_sb[:], AF.Exp, scale=-1.0)
    den_sb = sb.tile([BS, O], F32, tag="den")
    nc.vector.tensor_scalar_add(den_sb[:], eneg_sb[:], 1.0)
    out_sb = sb.tile([BS, O], F32, tag="out_sb")
    nc.vector.reciprocal(out_sb[:], den_sb[:])
    nc.sync.dma_start(io["out"].ap(), out_sb[:])


INPUT_SPECS = {
    "p_matrix": ([BS, S], BF16),
    "exer_emb": ([S, D], BF16),
    "exer_lam": ([S, 1], F32),
    "concept_emb": ([C, D], BF16),
    "resp_emb": ([3, D], F32),
    "Q_matrix": ([S, C], BF16),
    "Wq": ([D, D], F32),
    "bq": ([D], F32),
    "Wk": ([D, D], BF16),
    "bk": ([D], F32),
    "Wv": ([D, D], F32),
    "bv": ([D], F32),
    "Wmap": ([D, O], BF16),
    "bmap": ([O], F32),
}

_cache = {}


def build_nc():
    if "nc" in _cache:
        return _cache["nc"]
    nc = bacc.Bacc("TRN2", target_bir_lowering=False, debug=False, num_devices=NCORES)
    io = {}
    for name, (shape, dt) in INPUT_SPECS.items():
        io[name] = nc.dram_tensor(name, shape, dt, kind="ExternalInput")
    io["out"] = nc.dram_tensor("out", [BS, O], F32, kind="ExternalOutput")
    with tile.TileContext(nc) as tc:
        with ExitStack() as ctx:
            build_kernel(ctx, tc, io)
    nc.compile()
    _cache["nc"] = nc
    return nc


def make_in_maps(inputs: dict) -> list:
    in_maps = []
    base = {}
    for name, (shape, dt) in INPUT_SPECS.items():
        if name == "p_matrix":
            continue
        arr = np.ascontiguousarray(inputs[name])
        if dt == BF16:
            arr = arr.astype(ml_dtypes.bfloat16)
        base[name] = arr
    p_all = np.asarray(inputs["p_matrix"]).astype(ml_dtypes.bfloat16)
    for i in range(NCORES):
        m = dict(base)
        m["p_matrix"] = np.ascontiguousarray(p_all[i * BS : (i + 1) * BS])
        in_maps.append(m)
    return in_maps


def kernel(**inputs) -> np.ndarray:
    nc = build_nc()
    in_maps = make_in_maps(inputs)
    res = run_bass_kernel_spmd(nc, in_maps, core_ids=list(range(NCORES)))
    return np.concatenate([res.results[i]["out"] for i in range(NCORES)], axis=0)


# revision 7
# speedup vs baseline: 1.8738x; 1.1262x over previous
"""Trainium2 Bass kernel for nn_AttentionEncoder (ragged_sequence).

Math collapse (validated vs the jax reference to ~1e-7):
  - k comes from exercise embeddings -> batch-independent K [S,D].
  - q/v come from resp_emb[p] with p in {0,1,2}; valid positions have
    p in {1,2}, so per batch the softmax rows take only 2 distinct
    values. The whole [B,H,S,S] attention collapses to
        E[r,h,t]   = exp(score(q_r, k_t) / sqrt(dh))          (batch-free)
        A[r,h,j,b] = sum_t E[r,h,t] * (p[b,t]==j)             (tiny matmul)
    and the masked-mean-pooled output becomes
        pooled[b,d] = G1[h(d),b]*v1[d] + G2[h(d),b]*v2[d]
    with G linear in A. Final: sigmoid(pooled @ Wmap + bmap).

Sharding: data-parallel over batch; 8 cores x 16 batch rows, tables and
weights replicated, no collectives.

Implementation notes:
  - Score path runs bf16 (scores ~1e-4, exp(score)~1; bf16 error here is
    ~1e-5 at the output vs the 2e-2 gate). Count/value path stays f32.
  - Inputs are packed host-side into a few contiguous DMAs, split across
    both HWDGE queues (sync + scalar). Constants (identity, head masks,
    row selector) ride along in the packs.
  - Counts come for free from a ones-column appended to E.
  - sigmoid(x) = 0.5*tanh(x/2)+0.5 -- tanh lives in the same ACT table
    set as exp, avoiding a second ~1.3us ACT_TABLE_LOAD.
"""

import sys

sys.path.insert(0, "/opt/trn_rl_repo")

import math
from contextlib import ExitStack

import ml_dtypes
import numpy as np

import concourse.bass as bass
import concourse.tile as tile
from concourse import bacc, mybir
from concourse.bass_utils import run_bass_kernel_spmd

B, S, D, C, O, H = 128, 512, 128, 128, 256, 4
DH = D // H  # 32
NCORES = 8
BS = B // NCORES  # 16 batch rows per core
KT = S // 128  # 4 sequence tiles

F32 = mybir.dt.float32
BF16 = mybir.dt.bfloat16
AF = mybir.ActivationFunctionType
ALU = mybir.AluOpType
AX = mybir.AxisListType

# pack3 column offsets (bf16): ce | wk | wm | identity | hmaskT
P3_CE, P3_WK, P3_WM, P3_ID, P3_HM, P3_END = 0, 128, 256, 512, 640, 648
# packf column offsets (f32): lam(4) | bq | bk | retT(3)
PF_LAM, PF_BQ, PF_BK, PF_RET, PF_END = 0, 4, 5, 6, 9


def build_kernel(ctx: ExitStack, tc: tile.TileContext, io: dict):
    nc = tc.nc
    sb = ctx.enter_context(tc.tile_pool(name="sb", bufs=1))
    ps = ctx.enter_context(tc.tile_pool(name="ps", bufs=4, space="PSUM"))
    psb = ctx.enter_context(tc.tile_pool(name="psb", bufs=2, space="PSUM"))
    psa = ctx.enter_context(tc.tile_pool(name="psa", bufs=1, space="PSUM"))

    # ---- DMAs: split across the two HWDGE engines (sync, scalar) ----
    q_sb = sb.tile([128, KT, C], BF16, tag="q")  # Q as [s%128, s//128, c]
    nc.sync.dma_start(q_sb[:], io["pack_q"].ap().rearrange("p (k c) -> p k c", k=KT))
    pf_sb = sb.tile([128, PF_END], F32, tag="pf")
    nc.sync.dma_start(pf_sb[:], io["packf"].ap())
    p_sb = sb.tile([BS, S], BF16, tag="p")
    nc.sync.dma_start(p_sb[:], io["p_matrix"].ap())
    bmapb_sb = sb.tile([BS, O], F32, tag="bmapb")
    nc.sync.dma_start(bmapb_sb[:], io["bmap"].ap().unsqueeze(0).to_broadcast([BS, O]))
    bv3_sb = sb.tile([3, 128], F32, tag="bv3")
    nc.sync.dma_start(bv3_sb[:], io["bv"].ap().unsqueeze(0).to_broadcast([3, 128]))

    ee_sb = sb.tile([128, KT, D], BF16, tag="ee")
    nc.scalar.dma_start(ee_sb[:], io["pack_ee"].ap().rearrange("p (k d) -> p k d", k=KT))
    pk3_sb = sb.tile([128, P3_END], BF16, tag="pk3")
    nc.scalar.dma_start(pk3_sb[:], io["pack3"].ap())
    wqv_sb = sb.tile([128, 256], F32, tag="wqv")
    nc.scalar.dma_start(wqv_sb[:], io["wqv"].ap())
    cst_sb = sb.tile([8, 136], BF16, tag="cst")
    nc.scalar.dma_start(cst_sb[:], io["cst"].ap())

    ce = pk3_sb[:, P3_CE:P3_WK]
    wk = pk3_sb[:, P3_WK:P3_WM]
    wm = pk3_sb[:, P3_WM:P3_ID]
    ident = pk3_sb[:, P3_ID:P3_HM]
    hmt = pk3_sb[:, P3_HM:P3_END]  # hmaskT [d, rh]
    lam = pf_sb[:, PF_LAM:PF_BQ]
    bq = pf_sb[:, PF_BQ:PF_BK]
    bk = pf_sb[:, PF_BK:PF_RET]
    ret = pf_sb[:, PF_RET:PF_END]  # resp_emb^T [d, r]
    hmask = cst_sb[:, 0:128]  # [jh, d]
    sel38 = cst_sb[0:3, 128:136]  # [r, jh] row selector

    # ---- q/v tables (early; only need small DMAs) ----
    qt_ps = ps.tile([D, 3], F32, tag="ps")
    nc.tensor.matmul(qt_ps[:], wqv_sb[:, 0:128], ret)
    qt_sb = sb.tile([D, 3], BF16, tag="qt3")
    nc.scalar.add(qt_sb[:], qt_ps[:], bq)
    # qpad[d, r*4+h] = qt[d, 1+r] * (h == d//32)
    qpad_sb = sb.tile([D, 8], BF16, tag="qpad")
    nc.vector.tensor_tensor(
        qpad_sb[:].rearrange("p (r h) -> p r h", r=2),
        qt_sb[:, 1:3].unsqueeze(2).to_broadcast([D, 2, 4]),
        hmt.rearrange("p (r h) -> p r h", r=2),
        op=ALU.mult,
    )
    vf_ps = ps.tile([3, 128], F32, tag="ps")
    nc.tensor.matmul(vf_ps[:], ret, wqv_sb[:, 128:256])  # resp_emb @ Wv
    vf_sb = sb.tile([3, 128], BF16, tag="vf")
    nc.vector.tensor_add(vf_sb[:], vf_ps[:], bv3_sb[:])
    vsel_ps = ps.tile([8, 128], F32, tag="ps")
    nc.tensor.matmul(vsel_ps[:], sel38, vf_sb[:])  # row j*4+h holds v_{j+1}
    vsel_sb = sb.tile([8, 128], BF16, tag="vsel")
    nc.vector.tensor_tensor(vsel_sb[:], vsel_ps[:], hmask, op=ALU.mult)

    # ---- transposed p -> indicator stacks [t, j*32+b] ----
    pt_sb = sb.tile([128, KT, BS], BF16, tag="pt")
    for k in range(KT):
        pp_ps = ps.tile([128, BS], BF16, tag="ps")
        nc.tensor.transpose(pp_ps[:], p_sb[:, k * 128 : (k + 1) * 128], ident[0:BS, 0:BS])
        if k % 2:
            nc.scalar.copy(pt_sb[:, k, :], pp_ps[:])
        else:
            nc.vector.tensor_copy(pt_sb[:, k, :], pp_ps[:])
    ms_all = sb.tile([128, KT, 64], F32, tag="ms_all")
    nc.gpsimd.memset(ms_all[:], 0.0)
    nc.vector.tensor_scalar(ms_all[:, :, 0:BS], pt_sb[:], 1.0, None, op0=ALU.is_equal)
    nc.vector.tensor_scalar(
        ms_all[:, :, 32 : 32 + BS], pt_sb[:], 2.0, None, op0=ALU.is_equal
    )

    # ---- K^T via one ce-stationary and one wk-stationary matmul ----
    rsum_sb = sb.tile([128, KT], F32, tag="rsum")
    nc.vector.reduce_sum(rsum_sb[:], q_sb[:], axis=AX.X)
    rrec_sb = sb.tile([128, KT], F32, tag="rrec")
    nc.vector.reciprocal(rrec_sb[:], rsum_sb[:])
    sc_sb = sb.tile([128, KT], F32, tag="sc")  # lam / rowsum(Q)
    nc.vector.tensor_mul(sc_sb[:], lam, rrec_sb[:])
    qs_sb = sb.tile([128, KT, C], BF16, tag="qs")
    nc.vector.tensor_tensor(
        qs_sb[:], q_sb[:], sc_sb[:].unsqueeze(2).to_broadcast([128, KT, C]), op=ALU.mult
    )
    qst_sb = sb.tile([C, S], BF16, tag="qst")  # (scaled Q)^T
    eet_sb = sb.tile([D, S], BF16, tag="eet")  # ee^T
    for k in range(KT):
        tq_ps = ps.tile([128, 128], BF16, tag="ps")
        nc.tensor.transpose(tq_ps[:], qs_sb[:, k, :], ident)
        if k % 2:
            nc.scalar.copy(qst_sb[:, k * 128 : (k + 1) * 128], tq_ps[:])
        else:
            nc.vector.tensor_copy(qst_sb[:, k * 128 : (k + 1) * 128], tq_ps[:])
        te_ps = ps.tile([128, 128], BF16, tag="ps")
        nc.tensor.transpose(te_ps[:], ee_sb[:, k, :], ident)
        if k % 2:
            nc.vector.tensor_copy(eet_sb[:, k * 128 : (k + 1) * 128], te_ps[:])
        else:
            nc.scalar.copy(eet_sb[:, k * 128 : (k + 1) * 128], te_ps[:])
    cm_ps = psb.tile([128, S], F32, tag="big")  # (lam*concept_mean)^T
    nc.tensor.matmul(cm_ps[:], ce, qst_sb[:])
    ext_sb = sb.tile([D, S], BF16, tag="ext")  # ex^T
    nc.vector.tensor_add(ext_sb[:], cm_ps[:], eet_sb[:])
    kt_ps = psb.tile([128, S], F32, tag="big")
    nc.tensor.matmul(kt_ps[:], wk, ext_sb[:])
    kt_sb = sb.tile([D, S], BF16, tag="kt")  # K^T with bias
    nc.vector.tensor_scalar_add(kt_sb[:], kt_ps[:], bk)

    # ---- E = exp(scores) [t, rh]; col 8 stays 1.0 to produce counts ----
    e_all = sb.tile([128, KT, 9], F32, tag="e_all")
    nc.gpsimd.memset(e_all[:], 1.0)
    for k in range(KT):
        st_ps = ps.tile([128, 8], F32, tag="ps")
        nc.tensor.matmul(st_ps[:], kt_sb[:, k * 128 : (k + 1) * 128], qpad_sb[:])
        nc.scalar.activation(
            e_all[:, k, 0:8], st_ps[:], AF.Exp, scale=1.0 / math.sqrt(DH)
        )

    # ---- A[j*32+b, r*4+h | counts] ----
    a_ps = psa.tile([64, 9], F32, tag="a_ps")
    for k in range(KT):
        nc.tensor.matmul(
            a_ps[:], ms_all[:, k, :], e_all[:, k, :], start=(k == 0), stop=(k == KT - 1)
        )
    a1_sb = sb.tile([BS, 9], F32, tag="a1_sb")
    nc.vector.tensor_copy(a1_sb[:], a_ps[0:BS, :])
    a2_sb = sb.tile([BS, 9], F32, tag="a2_sb")
    nc.scalar.copy(a2_sb[:], a_ps[32 : 32 + BS, :])

    # ---- G weights [b, j*4+h] (col 8 of w0/w1 = c1n/c2n) ----
    zc_sb = sb.tile([BS, 9], F32, tag="zc")
    nc.vector.scalar_tensor_tensor(
        zc_sb[:], a1_sb[:], 1e-30, a2_sb[:], op0=ALU.max, op1=ALU.add
    )
    rz_sb = sb.tile([BS, 9], F32, tag="rz")
    nc.vector.reciprocal(rz_sb[:], zc_sb[:])
    w0_sb = sb.tile([BS, 9], F32, tag="w0")
    nc.vector.tensor_mul(w0_sb[:], a1_sb[:], rz_sb[:])
    w1_sb = sb.tile([BS, 9], F32, tag="w1")
    nc.vector.tensor_mul(w1_sb[:], a2_sb[:], rz_sb[:])
    gcat_sb = sb.tile([BS, 8], BF16, tag="gcat")
    tmp0_sb = sb.tile([BS, 4], F32, tag="tmp0")
    nc.vector.tensor_scalar_mul(tmp0_sb[:], w0_sb[:, 4:8], w1_sb[:, 8:9])
    nc.vector.scalar_tensor_tensor(
        gcat_sb[:, 0:4], w0_sb[:, 0:4], w0_sb[:, 8:9], tmp0_sb[:], op0=ALU.mult, op1=ALU.add
    )
    tmp1_sb = sb.tile([BS, 4], F32, tag="tmp1")
    nc.vector.tensor_scalar_mul(tmp1_sb[:], w1_sb[:, 4:8], w1_sb[:, 8:9])
    nc.vector.scalar_tensor_tensor(
        gcat_sb[:, 4:8], w1_sb[:, 0:4], w0_sb[:, 8:9], tmp1_sb[:], op0=ALU.mult, op1=ALU.add
    )

    # ---- pooled^T then the output matmul + tanh-sigmoid epilogue ----
    gt_ps = ps.tile([8, BS], BF16, tag="ps")
    nc.tensor.transpose(gt_ps[:], gcat_sb[:], ident[0:BS, 0:BS])
    gt_sb = sb.tile([8, BS], BF16, tag="gt")
    nc.vector.tensor_copy(gt_sb[:], gt_ps[:])
    ptd_ps = ps.tile([D, BS], F32, tag="ps")
    nc.tensor.matmul(ptd_ps[:], vsel_sb[:], gt_sb[:])  # vsel.T @ gt = pooled^T
    pt2_sb = sb.tile([D, BS], BF16, tag="pt2")
    nc.vector.tensor_copy(pt2_sb[:], ptd_ps[:])
    o_ps = ps.tile([BS, O], F32, tag="ps")
    nc.tensor.matmul(o_ps[:], pt2_sb[:], wm)
    logit_sb = sb.tile([BS, O], F32, tag="logit")
    nc.vector.tensor_add(logit_sb[:], o_ps[:], bmapb_sb[:])
    th_sb = sb.tile([BS, O], F32, tag="th")
    nc.scalar.activation(th_sb[:], logit_sb[:], AF.Tanh, scale=0.5)
    out_sb = sb.tile([BS, O], F32, tag="out_sb")
    nc.vector.tensor_scalar(out_sb[:], th_sb[:], 0.5, 0.5, op0=ALU.mult, op1=ALU.add)
    nc.sync.dma_start(io["out"].ap(), out_sb[:])


INPUT_SPECS = {
    "pack_q": ([128, S], BF16),
    "pack_ee": ([128, S], BF16),
    "pack3": ([128, P3_END], BF16),
    "wqv": ([128, 256], F32),
    "packf": ([128, PF_END], F32),
    "cst": ([8, 136], BF16),
    "p_matrix": ([BS, S], BF16),
    "bmap": ([O], F32),
    "bv": ([D], F32),
}

_cache = {}


def build_nc():
    if "nc" in _cache:
        return _cache["nc"]
    nc = bacc.Bacc("TRN2", target_bir_lowering=False, debug=False, num_devices=NCORES)
    io = {}
    for name, (shape, dt) in INPUT_SPECS.items():
        io[name] = nc.dram_tensor(name, shape, dt, kind="ExternalInput")
    io["out"] = nc.dram_tensor("out", [BS, O], F32, kind="ExternalOutput")
    with tile.TileContext(nc) as tc:
        with ExitStack() as ctx:
            build_kernel(ctx, tc, io)
    nc.compile()
    _cache["nc"] = nc
    return nc


def make_in_maps(inputs: dict) -> list:
    bf = ml_dtypes.bfloat16
    f32 = np.float32

    def seqpack(x):  # [S, D] -> [128, S] with s = k*128 + p
        return np.ascontiguousarray(
            np.asarray(x, f32).reshape(KT, 128, -1).transpose(1, 0, 2).reshape(128, -1)
        )

    pack_q = seqpack(inputs["Q_matrix"]).astype(bf)
    pack_ee = seqpack(inputs["exer_emb"]).astype(bf)

    d_idx = np.arange(D)
    rh = np.arange(8)
    hmaskT = (d_idx[:, None] // DH == rh[None, :] % 4).astype(f32)  # [d, rh]
    pack3 = np.concatenate(
        [
            np.asarray(inputs["concept_emb"], f32),
            np.asarray(inputs["Wk"], f32),
            np.asarray(inputs["Wmap"], f32),
            np.eye(128, dtype=f32),
            hmaskT,
        ],
        axis=1,
    ).astype(bf)
    wqv = np.ascontiguousarray(
        np.concatenate([np.asarray(inputs["Wq"], f32), np.asarray(inputs["Wv"], f32)], axis=1)
    )
    lam_cols = np.asarray(inputs["exer_lam"], f32).reshape(KT, 128).T  # [128, 4]
    packf = np.ascontiguousarray(
        np.concatenate(
            [
                lam_cols,
                np.asarray(inputs["bq"], f32)[:, None],
                np.asarray(inputs["bk"], f32)[:, None],
                np.asarray(inputs["resp_emb"], f32).T,
            ],
            axis=1,
        )
    )
    hmask = (rh[:, None] % 4 == np.arange(128)[None, :] // DH).astype(f32)  # [jh, d]
    sel38 = np.zeros((8, 8), f32)
    for r in range(3):
        for c in range(8):
            if r == 1 + c // 4:
                sel38[r, c] = 1.0
    cst = np.concatenate([hmask, sel38], axis=1).astype(bf)

    base = {
        "pack_q": pack_q,
        "pack_ee": pack_ee,
        "pack3": np.ascontiguousarray(pack3),
        "wqv": wqv,
        "packf": packf,
        "cst": np.ascontiguousarray(cst),
        "bmap": np.ascontiguousarray(np.asarray(inputs["bmap"], f32)),
        "bv": np.ascontiguousarray(np.asarray(inputs["bv"], f32)),
    }
    p_all = np.asarray(inputs["p_matrix"]).astype(bf)
    in_maps = []
    for i in range(NCORES):
        m = dict(base)
        m["p_matrix"] = np.ascontiguousarray(p_all[i * BS : (i + 1) * BS])
        in_maps.append(m)
    return in_maps


def kernel(**inputs) -> np.ndarray:
    nc = build_nc()
    in_maps = make_in_maps(inputs)
    res = run_bass_kernel_spmd(nc, in_maps, core_ids=list(range(NCORES)))
    return np.concatenate([res.results[i]["out"] for i in range(NCORES)], axis=0)
